# revision 1
# baseline (speedup 1.0000x reference)
"""Distributed sparse embedding lookup (mean combiner) on 8 Trainium2 cores.

Strategy (data-parallel over output rows, table replicated on every core):
  - Each core owns 1/8 of the output rows (13312 = 104*128). row_indices is
    sorted, so each core's keys are a contiguous slice of the input.
  - Keys are bucketed into 31 vocab windows of 32768 rows (dma_gather index
    tensors are int16). Within a window, keys are split into column-aligned
    chunks such that NO chunk contains two keys of the same output row
    (dma_scatter_add loses updates on duplicate targets within one
    instruction - HW-verified), distributing each row's in-window keys
    round-robin over the window's chunks.
  - Device pipeline per window: dma_gather (random 256B table rows, HBM ->
    SBUF) -> DVE multiply by per-key 1/count (mean pre-scaling, 0-stride
    broadcast along the 64-dim) -> per chunk one dma_scatter_add in
    SBUF-destination parity mode into one of two accumulator pairs
    (alternating, so the WAW serialization chains halve). Accumulator
    layout: output row r -> partition r%128, slot r//128; even slots in
    acc_a*, odd slots in acc_b* of the pair.
  - Final merge: pair0 + pair1 per parity on DVE, then two strided dense
    DMAs into the [13312, 64] output. Host concatenates the 8 core outputs.

All index preprocessing is host-side numpy; all table-data movement and
floating-point arithmetic run on the device.
"""
import numpy as np

_B, _S, _D = 4096, 26, 64
_V = 1_000_000
_M = 8
_R = _B * _S            # 106496 output rows
_RC = _R // _M          # 13312 rows per core = 104 slots * 128
_WIN = 32768
_NWIN = (_V + _WIN - 1) // _WIN      # 31
_ORC = _RC + 128        # +128 pad rows; pads scatter-add into row _RC
_NSLOT = _ORC // 128                 # 105 slots (even: 53, odd: 52)
_BG = 1024              # max num_idxs per dma_gather (HW ring validated)
_BS = 768               # max num_idxs per dma_scatter_add (HW-validated)
_NPAIR = 4              # accumulator pairs (independent WAW chains)

_prog_cache = {}


def _cdiv(a, b):
    return (a + b - 1) // b


def _pack16(v, budget, pad):
    out = np.full(budget, pad, dtype=v.dtype)
    out[: len(v)] = v
    return np.tile(out.reshape(-1, 16).T, (8, 1))


def _pack128(v, budget, pad):
    out = np.full(budget, pad, dtype=v.dtype)
    out[: len(v)] = v
    return out.reshape(-1, 128).T


def _chunk_window(keys, rows, invc, n_chunks, cap):
    """Distribute one window's keys into n_chunks lists, no row repeated
    within a chunk and no chunk above cap. keys are row-major; same-row keys
    are adjacent. Returns None if infeasible with this n_chunks."""
    out_k = [[] for _ in range(n_chunks)]
    out_r = [[] for _ in range(n_chunks)]
    out_i = [[] for _ in range(n_chunks)]
    fill = [0] * n_chunks
    n = len(keys)
    i = 0
    nxt = 0
    while i < n:
        j = i
        r = rows[i]
        while j < n and rows[j] == r:
            j += 1
        used = []
        for t in range(i, j):
            c = None
            for probe in range(n_chunks):
                cand = (nxt + t - i + probe) % n_chunks
                if fill[cand] < cap and cand not in used:
                    c = cand
                    break
            if c is None:
                return None
            used.append(c)
            out_k[c].append(keys[t])
            out_r[c].append(r)
            out_i[c].append(invc[t])
            fill[c] += 1
        nxt = (nxt + 1) % n_chunks
        i = j
    return out_k, out_r, out_i


def _prep(values, row_indices):
    """Returns (gather_budgets, chunk_budgets, in_maps)."""
    values = np.asarray(values).astype(np.int64)
    row_indices = np.asarray(row_indices).astype(np.int64)
    if np.any(np.diff(row_indices) < 0):
        order = np.argsort(row_indices, kind="stable")
        values, row_indices = values[order], row_indices[order]
    bounds = np.searchsorted(row_indices, np.arange(_M + 1) * _RC)
    per_core = []       # per core: per window: (keys, rows, invc)
    for c in range(_M):
        lo, hi = bounds[c], bounds[c + 1]
        keys = values[lo:hi]
        rows = row_indices[lo:hi] - c * _RC
        counts = np.bincount(rows, minlength=_RC).astype(np.float32)
        invc = (1.0 / np.maximum(counts, 1.0))[rows].astype(np.float32)
        # sort by (window, row): row-major within each window
        w = keys // _WIN
        order = np.lexsort((rows, w))
        ks, rs, iv = keys[order], rows[order], invc[order]
        wb = np.searchsorted(ks // _WIN, np.arange(_NWIN + 1))
        wins = []
        for wi in range(_NWIN):
            sl = slice(wb[wi], wb[wi + 1])
            wins.append((ks[sl] - wi * _WIN, rs[sl], iv[sl]))
        per_core.append(wins)

    # per window: number of chunks (same for all cores)
    n_chunks_w = []
    for wi in range(_NWIN):
        need = 1
        for c in range(_M):
            k, r, iv = per_core[c][wi]
            need = max(need, _cdiv(len(k), _BS))
            if len(r):
                _un, cnt = np.unique(r, return_counts=True)
                need = max(need, int(cnt.max()))
        n_chunks_w.append(need)

    # distribute into chunks; chunk budgets = max fill over cores, x128.
    # Raise n_chunks until every core fits the per-instruction cap.
    per_core_chunks = [[None] * _NWIN for _ in range(_M)]
    for wi in range(_NWIN):
        while True:
            ok = True
            for c in range(_M):
                k, r, iv = per_core[c][wi]
                res = _chunk_window(k, r, iv, n_chunks_w[wi], _BS)
                if res is None:
                    ok = False
                    break
                per_core_chunks[c][wi] = res
            if ok:
                break
            n_chunks_w[wi] += 1
    chunk_budgets = []   # flat list over (window, chunk)
    for wi in range(_NWIN):
        for ci in range(n_chunks_w[wi]):
            mx = max(len(per_core_chunks[c][wi][0][ci]) for c in range(_M))
            chunk_budgets.append((wi, max(_cdiv(mx, 128), 1) * 128))

    in_maps = []
    for c in range(_M):
        g_parts, s_parts, i_parts = [], [], []
        ptr = {wi: 0 for wi in range(_NWIN)}
        for wi, bud in chunk_budgets:
            ci = ptr[wi]
            ptr[wi] += 1
            ck, cr, ci_v = per_core_chunks[c][wi]
            k = np.asarray(ck[ci], np.int16)
            r = np.asarray(cr[ci], np.int16)
            iv = np.asarray(ci_v[ci], np.float32)
            g_parts.append(_pack16(k, bud, np.int16(0)))
            s_parts.append(_pack16(r, bud, np.int16(_RC)))  # pad -> dedicated pad slot
            i_parts.append(_pack128(iv, bud, np.float32(0.0)))   # zero contribution
        in_maps.append({
            "gidx": np.ascontiguousarray(np.concatenate(g_parts, axis=1)),
            "sidx": np.ascontiguousarray(np.concatenate(s_parts, axis=1)),
            "invc": np.ascontiguousarray(np.concatenate(i_parts, axis=1)),
        })
    return chunk_budgets, in_maps


def _build(chunk_budgets, n_reps=1):
    from concourse import bacc, mybir, tile

    nc = bacc.Bacc(None, target_bir_lowering=False, debug=False,
                   num_swdge_queues=1)
    table = nc.dram_tensor("table", [_V, _D], mybir.dt.float32,
                           kind="ExternalInput")
    gtot = sum(b // 16 for _w, b in chunk_budgets)
    ntot = sum(b // 128 for _w, b in chunk_budgets)
    gidx = nc.dram_tensor("gidx", [128, gtot], mybir.dt.int16,
                          kind="ExternalInput")
    sidx = nc.dram_tensor("sidx", [128, gtot], mybir.dt.int16,
                          kind="ExternalInput")
    invc = nc.dram_tensor("invc", [128, ntot], mybir.dt.float32,
                          kind="ExternalInput")
    out = nc.dram_tensor("out", [_ORC, _D], mybir.dt.float32,
                         kind="ExternalOutput")
    HGA = (_NSLOT + 1) // 2   # even-slot groups (incl. pad slot)
    HGB = _NSLOT // 2         # odd-slot groups

    with tile.TileContext(nc) as tc:
        with (
            tc.tile_pool(name="acc", bufs=1) as apool,
            tc.tile_pool(name="data", bufs=6) as dpool,
            tc.tile_pool(name="meta", bufs=1) as mpool,
        ):
            accs = []
            for p in range(_NPAIR):
                aa = apool.tile([128, HGA, _D], mybir.dt.float32, tag=f"aa{p}")
                ab = apool.tile([128, HGA, _D], mybir.dt.float32, tag=f"ab{p}")
                nc.vector.memset(aa[:], 0.0)
                nc.vector.memset(ab[:], 0.0)
                accs.append((aa, ab))

            # group consecutive same-window chunks into one gather of <= _BG
            ggroups = []
            for wi, bud in chunk_budgets:
                if (ggroups and ggroups[-1][0] == wi
                        and ggroups[-1][1] + bud <= _BG):
                    ggroups[-1][1] += bud
                    ggroups[-1][2].append(bud)
                else:
                    ggroups.append([wi, bud, [bud]])

            # preload all index/scale metadata once; slice on-chip
            gix = mpool.tile([128, gtot], mybir.dt.int16, tag="gix")
            six = mpool.tile([128, gtot], mybir.dt.int16, tag="six")
            ivx = mpool.tile([128, ntot], mybir.dt.float32, tag="ivx")
            nc.sync.dma_start(out=gix[:], in_=gidx[:])
            nc.sync.dma_start(out=six[:], in_=sidx[:])
            nc.sync.dma_start(out=ivx[:], in_=invc[:])

            for _rep in range(n_reps):
                goff = noff = 0
                chain = 0
                for wi, total, buds in ggroups:
                    nt = total // 128
                    base = wi * _WIN
                    wsize = min(_WIN, _V - base)
                    gat = dpool.tile([128, nt, _D], mybir.dt.float32, tag="gat")
                    nc.gpsimd.dma_gather(
                        out_ap=gat[:], in_ap=table[base:base + wsize, :],
                        idxs_ap=gix[:, goff:goff + total // 16],
                        num_idxs=total, num_idxs_reg=total,
                        elem_size=_D, queue_num=0,
                    )
                    sc = dpool.tile([128, nt, _D], mybir.dt.float32, tag="sc")
                    nc.vector.tensor_tensor(
                        out=sc[:], in0=gat[:],
                        in1=ivx[:, noff:noff + nt, None].to_broadcast(
                            [128, nt, _D]),
                        op=mybir.AluOpType.mult,
                    )
                    coff = 0
                    for bud in buds:
                        aa, ab = accs[chain % _NPAIR]
                        chain += 1
                        nc.gpsimd.dma_scatter_add(
                            out_ap=aa[:], in_ap=sc[:, coff:coff + bud // 128, :],
                            idxs_ap=six[:, goff + coff * 8:
                                        goff + coff * 8 + bud // 16],
                            num_idxs=bud, num_idxs_reg=bud,
                            elem_size=_D, queue_num=0, sbuf_tokens_per_rank=128,
                            parity_reg=0, out_ap_other=ab[:],
                        )
                        coff += bud // 128
                    goff += total // 16
                    noff += nt

            # merge pairs in place into accs[0] and write out
            for par in range(2):
                hg = HGA if par == 0 else HGB
                acc0 = accs[0][par][:, :hg, :]
                for p in range(1, _NPAIR):
                    nc.vector.tensor_add(out=acc0, in0=acc0,
                                         in1=accs[p][par][:, :hg, :])
                out_view = out[:].rearrange("(s p) d -> p s d", p=128)
                nc.sync.dma_start(out=out_view[:, par::2, :], in_=acc0)
    nc.compile()
    return nc


def _state(values, row_indices, emb_table, n_reps=1):
    chunk_budgets, in_maps = _prep(values, row_indices)
    key = (tuple(chunk_budgets), n_reps)
    if key not in _prog_cache:
        _prog_cache[key] = _build(chunk_budgets, n_reps=n_reps)
    nc = _prog_cache[key]
    table = np.ascontiguousarray(np.asarray(emb_table, dtype=np.float32))
    for m in in_maps:
        m["table"] = table
    return nc, in_maps


def kernel(values, row_indices, emb_table):
    from concourse.bass_utils import run_bass_kernel_spmd

    nc, in_maps = _state(values, row_indices, emb_table)
    res = run_bass_kernel_spmd(nc, in_maps, core_ids=list(range(_M)))
    full = np.concatenate(
        [np.asarray(res.results[c]["out"])[:_RC] for c in range(_M)], axis=0)
    return np.ascontiguousarray(full.reshape(_B, _S, _D))



# revision 2
# speedup vs baseline: 1.4229x; 1.4229x over previous
"""Distributed sparse embedding lookup (mean combiner) on 8 Trainium2 cores.

v3 design (data-parallel over output rows, bf16 table replicated):
  - Table uploaded as bf16 rows padded to 256B: block w = 32767 vocab rows
    + 1 zero entry (pad target), so int16 gather indices cover a window and
    pads gather exact zeros. 31 windows.
  - Per window, keys are grouped by output row and laid out in LEVEL runs:
    L0 = first key of every row (rows ordered by descending in-window
    count), Lk = (k+1)-th key of rows with > k keys, each run 128-aligned.
    Because every Lk lists rows in the same order as L0's prefix, folding
    duplicates is a handful of plain slot-range DVE adds:
        L0[0:nk] += Lk   (zeros in pad slots keep this exact).
  - After folding, each window holds one value per distinct row -> ONE
    dma_scatter_add per window (31 scatter instructions total; the cost
    model prices a scatter at a flat ~2.8us regardless of num_idxs).
  - Scatter elem = 128 bf16 units (whole padded entry) into double-wide
    bf16 parity accumulators; the junk half accumulates zeros.
  - Mean division happens once at the end: merged fp32 tile is multiplied
    by per-row reciprocal counts (one broadcast DVE op), then written out
    with a single dense DMA.
"""
import numpy as np
import ml_dtypes

_B, _S, _D = 4096, 26, 64
_V = 1_000_000
_M = 8
_R = _B * _S            # 106496 output rows
_RC = _R // _M          # 13312 rows per core
_WIN = 32767            # vocab rows per int16 window (+1 zero entry)
_NWIN = (_V + _WIN - 1) // _WIN      # 31
_ORC = _RC + 128        # +128 pad rows; pads scatter-add into row _RC
_NSLOT = _ORC // 128    # 105 slots (even: 53, odd: 52)
_BG = 1024              # max num_idxs per dma_gather (HW validated)
_BS = 3072              # max num_idxs per dma_scatter_add (HW validated)
_NPAIR = 2              # accumulator pairs

_prog_cache = {}


def _cdiv(a, b):
    return (a + b - 1) // b


def _pack16(v, budget, pad):
    out = np.full(budget, pad, dtype=v.dtype)
    out[: len(v)] = v
    return np.tile(out.reshape(-1, 16).T, (8, 1))


def _prep(values, row_indices):
    """Build per-core level-run layouts.

    Returns (win_meta, in_maps):
      win_meta: per window: dict(wi, lvl_slots=[slots per level], l0_n=max
        true distinct rows across cores, l0_slots, wlen)
      in_maps: per-core dict(gidx, sidx, recip).
    """
    values = np.asarray(values).astype(np.int64)
    row_indices = np.asarray(row_indices).astype(np.int64)
    if np.any(np.diff(row_indices) < 0):
        order = np.argsort(row_indices, kind="stable")
        values, row_indices = values[order], row_indices[order]
    bounds = np.searchsorted(row_indices, np.arange(_M + 1) * _RC)

    # per core, per window: list of levels, each level = (idxs, rows)
    core_levels = []
    recips = []
    for c in range(_M):
        lo, hi = bounds[c], bounds[c + 1]
        keys = values[lo:hi]
        rows = row_indices[lo:hi] - c * _RC
        counts = np.bincount(rows, minlength=_RC).astype(np.float32)
        recip = 1.0 / np.maximum(counts, 1.0)
        # recip laid out [128, _NSLOT]: row r -> partition r%128, slot r//128
        rp = np.ones((128, _NSLOT), np.float32)
        rp[:, :_RC // 128] = recip.reshape(_RC // 128, 128).T
        recips.append(rp)

        w = keys // _WIN
        order = np.lexsort((rows, w))
        ks, rs = keys[order], rows[order]
        wb = np.searchsorted(ks // _WIN, np.arange(_NWIN + 1))
        wins = []
        for wi in range(_NWIN):
            sl = slice(wb[wi], wb[wi + 1])
            k = ks[sl] - wi * _WIN     # in-window vocab idx [0, _WIN)
            r = rs[sl]
            # group by row; order rows by (-count, row)
            urow, start, cnt = np.unique(r, return_index=True,
                                         return_counts=True)
            ordr = np.lexsort((urow, -cnt))
            urow, start, cnt = urow[ordr], start[ordr], cnt[ordr]
            cmax = int(cnt.max()) if len(cnt) else 0
            levels = []
            for lv in range(cmax):
                m = cnt > lv
                levels.append((k[start[m] + lv], urow[m]))
            wins.append(levels)
        core_levels.append(wins)

    win_meta = []
    for wi in range(_NWIN):
        wlen = min(_WIN, _V - wi * _WIN) + 1     # + zero entry
        nlev = max(len(core_levels[c][wi]) for c in range(_M))
        lvl_n = [max((len(core_levels[c][wi][lv][0])
                      if lv < len(core_levels[c][wi]) else 0)
                     for c in range(_M)) for lv in range(nlev)]
        lvl_slots = [max(_cdiv(n, 128), 1) for n in lvl_n]
        win_meta.append(dict(wi=wi, lvl_slots=lvl_slots, l0_n=lvl_n[0] if lvl_n else 1,
                             wlen=wlen))

    in_maps = []
    for c in range(_M):
        g_parts, s_parts = [], []
        for meta in win_meta:
            wi = meta["wi"]
            zpad = np.int16(meta["wlen"] - 1)    # the zero entry
            levels = core_levels[c][wi]
            for lv, slots in enumerate(meta["lvl_slots"]):
                idxs = (levels[lv][0].astype(np.int16)
                        if lv < len(levels) else np.zeros(0, np.int16))
                g_parts.append(_pack16(idxs, slots * 128, zpad))
            rows0 = (levels[0][1].astype(np.int16)
                     if levels else np.zeros(0, np.int16))
            s_parts.append(_pack16(rows0, meta["lvl_slots"][0] * 128,
                                   np.int16(_RC)))
        in_maps.append({
            "gidx": np.ascontiguousarray(np.concatenate(g_parts, axis=1)),
            "sidx": np.ascontiguousarray(np.concatenate(s_parts, axis=1)),
            "recip": np.ascontiguousarray(recips[c]),
        })
    return win_meta, in_maps


def _build(win_meta, n_reps=1):
    from concourse import bacc, mybir, tile

    nc = bacc.Bacc(None, target_bir_lowering=False, debug=False,
                   num_swdge_queues=1)
    tlen = sum(m["wlen"] for m in win_meta)
    table = nc.dram_tensor("table", [tlen, 2 * _D], mybir.dt.bfloat16,
                           kind="ExternalInput")
    gtot = sum(sum(m["lvl_slots"]) * 8 for m in win_meta)
    stot = sum(m["lvl_slots"][0] * 8 for m in win_meta)
    gidx = nc.dram_tensor("gidx", [128, gtot], mybir.dt.int16,
                          kind="ExternalInput")
    sidx = nc.dram_tensor("sidx", [128, stot], mybir.dt.int16,
                          kind="ExternalInput")
    recip = nc.dram_tensor("recip", [128, _NSLOT], mybir.dt.float32,
                           kind="ExternalInput")
    out = nc.dram_tensor("out", [_ORC, _D], mybir.dt.float32,
                         kind="ExternalOutput")
    HGA = (_NSLOT + 1) // 2   # 53
    HGB = _NSLOT // 2         # 52

    with tile.TileContext(nc) as tc:
        with (
            tc.tile_pool(name="acc", bufs=1) as apool,
            tc.tile_pool(name="data", bufs=3) as dpool,
            tc.tile_pool(name="meta", bufs=1) as mpool,
        ):
            accs = []
            for p in range(_NPAIR):
                aa = apool.tile([128, HGA, _D], mybir.dt.bfloat16,
                                tag=f"aa{p}")
                ab = apool.tile([128, HGA, _D], mybir.dt.bfloat16,
                                tag=f"ab{p}")
                nc.vector.memset(aa[:], 0.0)
                nc.vector.memset(ab[:], 0.0)
                accs.append((aa, ab))

            gix = mpool.tile([128, gtot], mybir.dt.int16, tag="gix")
            six = mpool.tile([128, stot], mybir.dt.int16, tag="six")
            rcp = mpool.tile([128, _NSLOT], mybir.dt.float32, tag="rcp")
            nc.sync.dma_start(out=gix[:], in_=gidx[:])
            nc.sync.dma_start(out=six[:], in_=sidx[:])
            nc.sync.dma_start(out=rcp[:], in_=recip[:])

            for _rep in range(n_reps):
                goff = soff = woff = 0
                chain = 0
                for meta in win_meta:
                    lvl_slots = meta["lvl_slots"]
                    wslots = sum(lvl_slots)
                    wa = dpool.tile([128, wslots, 2 * _D],
                                    mybir.dt.bfloat16, tag="wa")
                    # gather slices of <= _BG idxs
                    so = 0
                    while so < wslots:
                        sn = min(_BG // 128, wslots - so)
                        nidx = sn * 128
                        nc.gpsimd.dma_gather(
                            out_ap=wa[:, so:so + sn, :],
                            in_ap=table[woff:woff + meta["wlen"], :],
                            idxs_ap=gix[:, goff + so * 8:
                                        goff + (so + sn) * 8],
                            num_idxs=nidx, num_idxs_reg=nidx,
                            elem_size=2 * _D, queue_num=0,
                        )
                        so += sn
                    # compact to the real 64-unit halves (drop the pad half)
                    cw = dpool.tile([128, wslots, _D], mybir.dt.bfloat16,
                                    tag="cw")
                    nc.vector.tensor_copy(out=cw[:], in_=wa[:, :, 0:_D])
                    # fold levels into L0 prefix
                    off = lvl_slots[0]
                    for lv in range(1, len(lvl_slots)):
                        ls = lvl_slots[lv]
                        nc.vector.tensor_tensor(
                            out=cw[:, 0:ls, :], in0=cw[:, 0:ls, :],
                            in1=cw[:, off:off + ls, :],
                            op=mybir.AluOpType.add,
                        )
                        off += ls
                    # one scatter per window (L0 rows are distinct)
                    n_idx = meta["l0_n"]
                    n_sl = lvl_slots[0]
                    st = 0
                    while n_idx > 0:
                        cur = min(n_idx, _BS)
                        cur_sl = min(_cdiv(cur, 128), n_sl - st)
                        aa, ab = accs[chain % _NPAIR]
                        chain += 1
                        nc.gpsimd.dma_scatter_add(
                            out_ap=aa[:], in_ap=cw[:, st:st + cur_sl, :],
                            idxs_ap=six[:, soff + st * 8:
                                        soff + (st + cur_sl) * 8],
                            num_idxs=cur, num_idxs_reg=cur,
                            elem_size=_D, queue_num=0,
                            sbuf_tokens_per_rank=128,
                            parity_reg=0, out_ap_other=ab[:],
                        )
                        n_idx -= cur
                        st += cur_sl
                    goff += wslots * 8
                    soff += lvl_slots[0] * 8
                    woff += meta["wlen"]

            # merge pairs (real halves only), scale by recip, one dense DMA
            mg = apool.tile([128, _NSLOT, _D], mybir.dt.float32, tag="mg")
            for par in range(2):
                hg = HGA if par == 0 else HGB
                dst = mg[:, par::2, :]
                nc.vector.tensor_tensor(
                    out=dst, in0=accs[0][par][:, :hg, :],
                    in1=accs[1][par][:, :hg, :],
                    op=mybir.AluOpType.add)
                for p in range(2, _NPAIR):
                    nc.vector.tensor_tensor(
                        out=dst, in0=dst, in1=accs[p][par][:, :hg, :],
                        op=mybir.AluOpType.add)
            nc.vector.tensor_tensor(
                out=mg[:], in0=mg[:],
                in1=rcp[:, :, None].to_broadcast([128, _NSLOT, _D]),
                op=mybir.AluOpType.mult)
            out_view = out[:].rearrange("(s p) d -> p s d", p=128)
            nc.sync.dma_start(out=out_view, in_=mg[:])
    nc.compile()
    return nc


def _table_blocks(emb_table):
    t = np.asarray(emb_table, dtype=np.float32)
    bf = ml_dtypes.bfloat16
    blocks = []
    for wi in range(_NWIN):
        lo = wi * _WIN
        hi = min(lo + _WIN, _V)
        blk = np.zeros((hi - lo + 1, 2 * _D), bf)
        blk[:-1, :_D] = t[lo:hi].astype(bf)
        blocks.append(blk)
    return np.ascontiguousarray(np.concatenate(blocks, axis=0))


def _state(values, row_indices, emb_table, n_reps=1):
    win_meta, in_maps = _prep(values, row_indices)
    key = (tuple(tuple(m["lvl_slots"]) + (m["l0_n"],) for m in win_meta),
           n_reps)
    if key not in _prog_cache:
        _prog_cache[key] = _build(win_meta, n_reps=n_reps)
    nc = _prog_cache[key]
    table = _table_blocks(emb_table)
    for m in in_maps:
        m["table"] = table
    return nc, in_maps


def kernel(values, row_indices, emb_table):
    from concourse.bass_utils import run_bass_kernel_spmd

    nc, in_maps = _state(values, row_indices, emb_table)
    res = run_bass_kernel_spmd(nc, in_maps, core_ids=list(range(_M)))
    full = np.concatenate(
        [np.asarray(res.results[c]["out"])[:_RC] for c in range(_M)], axis=0)
    return np.ascontiguousarray(full.reshape(_B, _S, _D))
